# revision 4
# baseline (speedup 1.0000x reference)
"""DCN CrossLayer kernel for Trainium2 (8 NeuronCores, batch-sharded).

Math: the reference loop
    cross = x
    for i in range(L):
        s_i   = sum(cross, axis=1)                  # (B, 1)
        cross = s_i * x * W[i] + b[i] + cross
collapses to
    out[b, k] = x[b, k] * (1 + sum_i s_i[b] * W[i, k]) + Bsum[k]
with
    u_i[b]  = sum_k x[b, k] * W[i, k]
    s_0[b]  = sum_k x[b, k]
    s_{i+1} = s_i * (1 + u_i) + beta_i,   beta_i = sum_k b[i, k]
    Bsum[k] = sum_i b[i, k]

Layout strategy (v2): the host uploads x PRE-TRANSPOSED and in fp16
(x^T: [D, rows], k on partitions).  This halves the input HBM bytes
(8 MiB -> 4 MiB per core) and removes all 128 big PE transposes the
natural layout needs: the k-contraction for [s_0, u_i] is a direct
PE matmul U = A^T @ x^T accumulated over 16 k-chunks, and the final
product is computed transposed, out^T = x^T * T^T with
T^T[k, b] = 1 + sum_i W[i, k] s'_i[b]  (one [5]-deep matmul per
[128 k x 512 b] chunk).  The host transposes the fp16 result back.

Per-core schedule: the 1024-row b-range splits into two 512-wide
halves.  Half 0's 16 k-chunk loads stream first, so its U/recursion/
T-multiply/store pipeline runs while half 1 loads; the final load
batches shrink to single chunks so U catches up during the stream.
The elementwise multiply (the only pass that must touch every output
element on a compute engine) is split DVE (reads T from PSUM
directly) / ACT-copy+GPSIMD / ACT-copy+DVE-fp16 so no single engine
paces the tail.  CoreSim DMA floor: 8 MiB @ 360 GB/s = 23.3 us.

Precision: fp16 x quantization ~2.4e-4, fp16 store ~2.4e-4, s'
chain ~3e-4 -> total rel err ~5e-4 (gate is 2e-2).
"""

import sys

sys.path.insert(0, "/opt/trn_rl_repo")

import numpy as np

import concourse.bacc as bacc
import concourse.tile as tile
from concourse import mybir
from concourse.bass_utils import run_bass_kernel_spmd
from concourse.masks import make_identity

N_CORES = 8
B, D, L = 8192, 2048, 4
RB = B // N_CORES            # 1024 batch rows per core
P = 128                      # partitions
KC = D // P                  # 16 k-chunks of 128
NQ = 4                       # b quarters per core
QW = RB // NQ                # 256 b columns per quarter
NSUB = QW // P               # 2 recursion subtiles per quarter

F32 = mybir.dt.float32
F16 = mybir.dt.float16
ADD = mybir.AluOpType.add
MULT = mybir.AluOpType.mult

# Chunk ranges per load DMA within a quarter.
LOAD_BATCHES = ((0, 8), (8, 16))
# Chunk ranges per store DMA within a quarter.
STORE_GROUPS = ((0, 8), (8, 16))
# Multiply path per chunk, per quarter: 'd' = DVE reads T from PSUM
# directly, 'g' = ACT copies T to fp16 SBUF + GPSIMD multiply.
PATHS = "dgdgdgdgdgdgdggg"


def build_program(betas):
    """Build the per-core Bass program (same program on all 8 cores)."""
    nc = bacc.Bacc("TRN2", target_bir_lowering=False)

    xt_d = nc.dram_tensor("xt", [D, RB], F16, kind="ExternalInput")
    a_d = nc.dram_tensor("acoef", [P, KC * L], F16, kind="ExternalInput")
    wv_d = nc.dram_tensor("wv", [L + 1, D], F16, kind="ExternalInput")
    out_d = nc.dram_tensor("out", [D, RB], F16, kind="ExternalOutput")

    xt_t = xt_d.rearrange("(c p) b -> p c b", p=P)
    out_t = out_d.rearrange("(c p) b -> p c b", p=P)

    with tile.TileContext(nc) as tc:
        with (
            tc.tile_pool(name="consts", bufs=1) as consts,
            tc.tile_pool(name="xp", bufs=1) as xp,
            tc.tile_pool(name="op", bufs=1) as op,
            tc.tile_pool(name="smalls", bufs=1) as smalls,
            tc.tile_pool(name="tsbp", bufs=4) as tsbp,
            tc.tile_pool(name="u_ps", bufs=1, space="PSUM") as u_ps,
            tc.tile_pool(name="st_ps", bufs=1, space="PSUM") as st_ps,
            tc.tile_pool(name="t_ps", bufs=4, space="PSUM") as t_ps,
        ):
            # x loads lead on the SP ring; tiny consts ride SWDGE (no HWDGE
            # slot) so they only displace ~150ns of the x stream.
            xall = xp.tile([P, KC, RB], F16, tag="x")
            for q in range(NQ):
                qs = slice(q * QW, (q + 1) * QW)
                for lo, hi in LOAD_BATCHES:
                    nc.sync.dma_start(
                        out=xall[:, lo:hi, qs], in_=xt_t[:, lo:hi, qs]
                    )
                if q == 0:
                    a_sb = consts.tile([P, KC * L], F16)
                    nc.gpsimd.dma_start(out=a_sb, in_=a_d[:])
                    wv_sb = consts.tile([L + 1, D], F16)
                    nc.gpsimd.dma_start(out=wv_sb, in_=wv_d[:])
                    ident = consts.tile([P, P], F32)
                    make_identity(nc, ident)

            oall = op.tile([P, KC, RB], F16, tag="o")
            # U tiles: rows 0..3 hold the U accumulation; after the ACT
            # evacuation the same PSUM region is reused as the transpose
            # target for the natural-layout u (saves PSUM, and the WAR dep
            # is exactly the evacuation).
            u_tiles = [
                u_ps.tile([P, QW], F32, tag=f"u{q % 2}", name=f"u{q % 2}")
                for q in range(NQ)
            ]
            st_tiles = [None] * NQ

            def u_mms(q, lo, hi):
                """U^T accumulation matmuls for chunks [lo, hi) of quarter q."""
                qs = slice(q * QW, (q + 1) * QW)
                for c in range(lo, hi):
                    nc.tensor.matmul(
                        u_tiles[q][:L, :],
                        a_sb[:, c * L : (c + 1) * L],
                        xall[:, c, qs],
                        start=(c == 0),
                        stop=(c == KC - 1),
                    )

            def recursion(q):
                """U -> S' for quarter q.

                The [4, 256] U rows live on partitions 1..3, which compute
                engines cannot address individually (mod-32 base rule), so
                transpose to natural [128, sub, i] layout, run the chain on
                GPSIMD (keeps DVE free for multiplies), transpose back.
                """
                u_sb = smalls.tile([L, QW], F32, tag=f"usb{q}")
                nc.scalar.copy(u_sb, u_tiles[q][:L, :])
                un_ps = u_tiles[q]
                for s in range(NSUB):
                    nc.tensor.transpose(
                        un_ps[:, s * L : (s + 1) * L],
                        u_sb[:, s * P : (s + 1) * P],
                        ident[:L, :L],
                    )
                un_v = un_ps[:, : NSUB * L].rearrange("p (s l) -> p s l", s=NSUB)
                sn = smalls.tile([P, NSUB, L + 1], F32, tag=f"sn{q}")
                nc.gpsimd.memset(sn[:, :, L], 1.0)
                nc.scalar.copy(sn[:, :, 0], un_v[:, :, 0])
                if all(bt == 0.0 for bt in betas):
                    # ACT evacuates 1+u_i in one fused op; the chain is then
                    # three plain multiplies, which Pool supports (keeps DVE
                    # free; TensorScalarPtr is not a Pool instruction).
                    un1 = smalls.tile([P, NSUB, L - 1], F32, tag=f"un{q}")
                    nc.scalar.add(un1, un_v[:, :, 1:], 1.0)
                    for i in range(L - 1):
                        nc.gpsimd.tensor_mul(
                            sn[:, :, i + 1], sn[:, :, i], un1[:, :, i]
                        )
                else:
                    for i in range(L - 1):
                        nc.vector.scalar_tensor_tensor(
                            out=sn[:, :, i + 1],
                            in0=un_v[:, :, i + 1],
                            scalar=1.0,
                            in1=sn[:, :, i],
                            op0=ADD,
                            op1=MULT,
                        )
                        nc.vector.tensor_scalar_add(
                            sn[:, :, i + 1], sn[:, :, i + 1], float(betas[i])
                        )
                stp = st_ps.tile([L + 1, QW], F32, tag=f"stp{q % 2}")
                for s in range(NSUB):
                    nc.tensor.transpose(
                        stp[:, s * P : (s + 1) * P], sn[:, s, :], ident
                    )
                st = smalls.tile([L + 1, QW], F16, tag=f"st{q}")
                nc.scalar.copy(st, stp)
                st_tiles[q] = st

            def v_mult(q, lo, hi):
                """T^T matmul + elementwise multiply for chunks [lo, hi)."""
                qs = slice(q * QW, (q + 1) * QW)
                for c in range(lo, hi):
                    tp = t_ps.tile([P, QW], F32, tag="t")
                    nc.tensor.matmul(
                        tp,
                        wv_sb[:, c * P : (c + 1) * P],
                        st_tiles[q],
                        start=True,
                        stop=True,
                    )
                    path = PATHS[c]
                    if path == "d":
                        nc.vector.tensor_mul(oall[:, c, qs], xall[:, c, qs], tp)
                    else:
                        tsb = tsbp.tile([P, QW], F16, tag="tsb")
                        nc.scalar.copy(tsb, tp)
                        eng = nc.gpsimd if path == "g" else nc.vector
                        eng.tensor_mul(oall[:, c, qs], xall[:, c, qs], tsb)

            def stores(q):
                qs = slice(q * QW, (q + 1) * QW)
                for lo, hi in STORE_GROUPS:
                    nc.sync.dma_start(
                        out=out_t[:, lo:hi, qs],
                        in_=oall[:, lo:hi, qs],
                    )

            # Emission order sets scheduler PRIORITY (the Tile list scheduler
            # pops the lowest-priority READY instruction per engine).  All
            # loads were emitted first so the in-order SP queue streams the
            # full input before the first store.  Compute is emitted per
            # quarter (U -> recursion -> T/multiply -> store) so PE never
            # blocks on a later quarter's data while an earlier quarter's
            # T matmuls are ready: outputs flow from ~5us and the store
            # stream tails the load stream with no DMA idle gap.
            for q in range(NQ):
                u_mms(q, 0, KC)
                recursion(q)
                v_mult(q, 0, KC)
                stores(q)

    nc.finalize()
    return nc


_CACHE = {}


def _get_program(betas):
    key = tuple(float(b) for b in betas)
    if key not in _CACHE:
        _CACHE[key] = build_program(key)
    return _CACHE[key]


def make_in_maps(x, W, b):
    """Shard x (fp16, transposed) across cores; replicate coefficients."""
    x = np.asarray(x, dtype=np.float32)
    W = np.asarray(W, dtype=np.float32)
    assert x.shape == (B, D) and W.shape == (L, D)

    x16 = x.astype(np.float16)
    # A = [ones, W0, W1, W2] as [P, KC*L]: a[p, c*L+i] = A[c*128+p, i]
    a_mat = np.concatenate([np.ones((D, 1), np.float32), W[: L - 1].T], axis=1)
    a_host = np.ascontiguousarray(
        a_mat.reshape(KC, P, L).transpose(1, 0, 2).reshape(P, KC * L)
    ).astype(np.float16)
    # W'' = [W; ones] as [L+1, D]
    wv_host = np.concatenate([W, np.ones((1, D), np.float32)], axis=0).astype(
        np.float16
    )
    return [
        {
            "xt": np.ascontiguousarray(x16[i * RB : (i + 1) * RB].T),
            "acoef": a_host,
            "wv": wv_host,
        }
        for i in range(N_CORES)
    ]


def kernel(**inputs) -> np.ndarray:
    x = np.asarray(inputs["x"], dtype=np.float32)
    W = np.asarray(inputs["W"], dtype=np.float32)
    b = np.asarray(inputs["b"], dtype=np.float32)

    betas = b.sum(axis=1, dtype=np.float64).astype(np.float32)
    nc = _get_program(betas)
    in_maps = make_in_maps(x, W, b)
    res = run_bass_kernel_spmd(nc, in_maps, list(range(N_CORES)))
    out = np.concatenate(
        [res.results[i]["out"].T for i in range(N_CORES)], axis=0
    ).astype(np.float32)

    bsum = b.sum(axis=0, dtype=np.float64).astype(np.float32)
    if np.any(bsum != 0.0):
        out = out + bsum[None, :]
    return out



# revision 7
# speedup vs baseline: 1.0784x; 1.0784x over previous
"""DCN CrossLayer kernel for Trainium2 (8 NeuronCores, batch-sharded).

Math: the reference loop
    cross = x
    for i in range(L):
        s_i   = sum(cross, axis=1)                  # (B, 1)
        cross = s_i * x * W[i] + b[i] + cross
collapses to
    out[b, k] = x[b, k] * (1 + sum_i s_i[b] * W[i, k]) + Bsum[k]
with
    u_i[b]  = sum_k x[b, k] * W[i, k]
    s_0[b]  = sum_k x[b, k]
    s_{i+1} = s_i * (1 + u_i) + beta_i,   beta_i = sum_k b[i, k]
    Bsum[k] = sum_i b[i, k]

Layout strategy (v2): the host uploads x PRE-TRANSPOSED and in fp16
(x^T: [D, rows], k on partitions).  This halves the input HBM bytes
(8 MiB -> 4 MiB per core) and removes all 128 big PE transposes the
natural layout needs: the k-contraction for [s_0, u_i] is a direct
PE matmul U = A^T @ x^T accumulated over 16 k-chunks, and the final
product is computed transposed, out^T = x^T * T^T with
T^T[k, b] = 1 + sum_i W[i, k] s'_i[b]  (one [5]-deep matmul per
[128 k x 512 b] chunk).  The host transposes the fp16 result back.

Per-core schedule: the 1024-row b-range splits into two 512-wide
halves.  Half 0's 16 k-chunk loads stream first, so its U/recursion/
T-multiply/store pipeline runs while half 1 loads; the final load
batches shrink to single chunks so U catches up during the stream.
The elementwise multiply (the only pass that must touch every output
element on a compute engine) is split DVE (reads T from PSUM
directly) / ACT-copy+GPSIMD / ACT-copy+DVE-fp16 so no single engine
paces the tail.  CoreSim DMA floor: 8 MiB @ 360 GB/s = 23.3 us.

Precision: fp16 x quantization ~2.4e-4, fp16 store ~2.4e-4, s'
chain ~3e-4 -> total rel err ~5e-4 (gate is 2e-2).
"""

import sys

sys.path.insert(0, "/opt/trn_rl_repo")

import numpy as np

import concourse.bacc as bacc
import concourse.tile as tile
from concourse import mybir
from concourse.bass_utils import run_bass_kernel_spmd
from concourse.masks import make_identity

N_CORES = 8
B, D, L = 8192, 2048, 4
RB = B // N_CORES            # 1024 batch rows per core
P = 128                      # partitions
KC = D // P                  # 16 k-chunks of 128
NQ = 4                       # b quarters per core
QW = RB // NQ                # 256 b columns per quarter
NSUB = QW // P               # 2 recursion subtiles per quarter

F32 = mybir.dt.float32
F16 = mybir.dt.float16
ADD = mybir.AluOpType.add
MULT = mybir.AluOpType.mult

NH = 2                       # b halves per core (multiply/store granularity)
HW = RB // NH                # 512 b columns per half

# Chunk ranges per load DMA within a quarter.
LOAD_BATCHES = ((0, 8), (8, 16))
# Chunk ranges per store DMA within a half.
STORE_GROUPS = ((0, 4), (4, 8), (8, 12), (12, 16))
# Multiply path per chunk, per half: 'd' = DVE reads T from PSUM
# directly, 'g' = ACT copies T to fp16 SBUF + GPSIMD multiply.
PATHS = "dgdgdgdgdgdgdgdg"


def build_program(betas):
    """Build the per-core Bass program (same program on all 8 cores)."""
    nc = bacc.Bacc("TRN2", target_bir_lowering=False)

    xt_d = nc.dram_tensor("xt", [D, RB], F16, kind="ExternalInput")
    a_d = nc.dram_tensor("acoef", [P, KC * L], F16, kind="ExternalInput")
    wv_d = nc.dram_tensor("wv", [L + 1, D], F16, kind="ExternalInput")
    out_d = nc.dram_tensor("out", [D, RB], F16, kind="ExternalOutput")

    xt_t = xt_d.rearrange("(c p) b -> p c b", p=P)
    out_t = out_d.rearrange("(c p) b -> p c b", p=P)

    with tile.TileContext(nc) as tc:
        with (
            tc.tile_pool(name="consts", bufs=1) as consts,
            tc.tile_pool(name="xp", bufs=1) as xp,
            tc.tile_pool(name="op", bufs=1) as op,
            tc.tile_pool(name="smalls", bufs=1) as smalls,
            tc.tile_pool(name="tsbp", bufs=4) as tsbp,
            tc.tile_pool(name="u_ps", bufs=1, space="PSUM") as u_ps,
            tc.tile_pool(name="st_ps", bufs=1, space="PSUM") as st_ps,
            tc.tile_pool(name="t_ps", bufs=4, space="PSUM") as t_ps,
        ):
            # x loads lead on the SP ring; tiny consts ride SWDGE (no HWDGE
            # slot) so they only displace ~150ns of the x stream.
            xall = xp.tile([P, KC, RB], F16, tag="x")
            for q in range(NQ):
                qs = slice(q * QW, (q + 1) * QW)
                for lo, hi in LOAD_BATCHES:
                    nc.sync.dma_start(
                        out=xall[:, lo:hi, qs], in_=xt_t[:, lo:hi, qs]
                    )
                if q == 0:
                    a_sb = consts.tile([P, KC * L], F16)
                    nc.gpsimd.dma_start(out=a_sb, in_=a_d[:])
                    wv_sb = consts.tile([L + 1, D], F16)
                    nc.gpsimd.dma_start(out=wv_sb, in_=wv_d[:])
                    ident = consts.tile([P, P], F32)
                    make_identity(nc, ident)

            oall = op.tile([P, KC, RB], F16, tag="o")
            # U tiles: rows 0..3 hold the U accumulation; after the ACT
            # evacuation the same PSUM region is reused as the transpose
            # target for the natural-layout u (saves PSUM, and the WAR dep
            # is exactly the evacuation).
            u_tiles = [
                u_ps.tile([P, QW], F32, tag=f"u{q % 2}", name=f"u{q % 2}")
                for q in range(NQ)
            ]
            st_tiles = [None] * NQ

            def u_mms(q, lo, hi):
                """U^T accumulation matmuls for chunks [lo, hi) of quarter q."""
                qs = slice(q * QW, (q + 1) * QW)
                for c in range(lo, hi):
                    nc.tensor.matmul(
                        u_tiles[q][:L, :],
                        a_sb[:, c * L : (c + 1) * L],
                        xall[:, c, qs],
                        start=(c == 0),
                        stop=(c == KC - 1),
                    )

            # Per-half S' staging: the two quarter recursions of half h
            # transpose into a shared [5, HW] PSUM tile; one ACT copy
            # evacuates it to fp16 SBUF for the T matmuls.
            stp_tiles = [
                st_ps.tile([L + 1, HW], F32, tag=f"stp{h}", name=f"stp{h}")
                for h in range(NH)
            ]
            st_tiles = [None] * NH

            def recursion(q):
                """U -> S' for quarter q (into half q//2's stp tile).

                The [4, 256] U rows live on partitions 1..3, which compute
                engines cannot address individually (mod-32 base rule), so
                transpose to natural [128, sub, i] layout, run the chain on
                GPSIMD (keeps DVE free for multiplies), transpose back.
                """
                u_sb = smalls.tile([L, QW], F32, tag=f"usb{q}")
                nc.scalar.copy(u_sb, u_tiles[q][:L, :])
                un_ps = u_tiles[q]
                for s in range(NSUB):
                    nc.tensor.transpose(
                        un_ps[:, s * L : (s + 1) * L],
                        u_sb[:, s * P : (s + 1) * P],
                        ident[:L, :L],
                    )
                un_v = un_ps[:, : NSUB * L].rearrange("p (s l) -> p s l", s=NSUB)
                sn = smalls.tile([P, NSUB, L + 1], F32, tag=f"sn{q}")
                nc.gpsimd.memset(sn[:, :, L], 1.0)
                nc.scalar.copy(sn[:, :, 0], un_v[:, :, 0])
                if all(bt == 0.0 for bt in betas):
                    # ACT evacuates 1+u_i in one fused op; the chain is then
                    # three plain multiplies, which Pool supports (keeps DVE
                    # free; TensorScalarPtr is not a Pool instruction).
                    un1 = smalls.tile([P, NSUB, L - 1], F32, tag=f"un{q}")
                    nc.scalar.add(un1, un_v[:, :, 1:], 1.0)
                    for i in range(L - 1):
                        nc.gpsimd.tensor_mul(
                            sn[:, :, i + 1], sn[:, :, i], un1[:, :, i]
                        )
                else:
                    for i in range(L - 1):
                        nc.vector.scalar_tensor_tensor(
                            out=sn[:, :, i + 1],
                            in0=un_v[:, :, i + 1],
                            scalar=1.0,
                            in1=sn[:, :, i],
                            op0=ADD,
                            op1=MULT,
                        )
                        nc.vector.tensor_scalar_add(
                            sn[:, :, i + 1], sn[:, :, i + 1], float(betas[i])
                        )
                stp = stp_tiles[q // 2]
                off = (q % 2) * QW
                for s in range(NSUB):
                    nc.tensor.transpose(
                        stp[:, off + s * P : off + (s + 1) * P], sn[:, s, :], ident
                    )

            def st_copy(h):
                """Evacuate half h's S' PSUM tile to fp16 SBUF."""
                st = smalls.tile([L + 1, HW], F16, tag=f"st{h}")
                nc.scalar.copy(st, stp_tiles[h])
                st_tiles[h] = st

            def v_mult(h, lo, hi):
                """T^T matmul + elementwise multiply for chunks [lo, hi)."""
                hs = slice(h * HW, (h + 1) * HW)
                for c in range(lo, hi):
                    tp = t_ps.tile([P, HW], F32, tag="t")
                    nc.tensor.matmul(
                        tp,
                        wv_sb[:, c * P : (c + 1) * P],
                        st_tiles[h],
                        start=True,
                        stop=True,
                    )
                    path = PATHS[c]
                    if path == "d":
                        nc.vector.tensor_mul(oall[:, c, hs], xall[:, c, hs], tp)
                    else:
                        tsb = tsbp.tile([P, HW], F16, tag="tsb")
                        nc.scalar.copy(tsb, tp)
                        eng = nc.gpsimd if path == "g" else nc.vector
                        eng.tensor_mul(oall[:, c, hs], xall[:, c, hs], tsb)

            def stores(h, groups):
                hs = slice(h * HW, (h + 1) * HW)
                for lo, hi in groups:
                    nc.sync.dma_start(
                        out=out_t[:, lo:hi, hs],
                        in_=oall[:, lo:hi, hs],
                    )

            # Emission order sets scheduler PRIORITY (the Tile list scheduler
            # pops the lowest-priority READY instruction per engine).  All
            # loads were emitted first so the in-order SP queue streams the
            # full input before the first store.  U/recursion run per
            # quarter so they overlap the load stream; the T-matmul/multiply
            # pipeline runs per half (512-wide) where engine fixed overheads
            # amortize.  Interleaving half-0's multiply emission with
            # quarter-2/3's U work keeps PE from blocking on later data
            # while earlier T matmuls are ready.
            u_mms(0, 0, KC)
            recursion(0)
            u_mms(1, 0, KC)
            recursion(1)
            st_copy(0)
            v_mult(0, 0, 8)
            stores(0, STORE_GROUPS[:2])
            u_mms(2, 0, KC)
            recursion(2)
            v_mult(0, 8, KC)
            stores(0, STORE_GROUPS[2:])
            u_mms(3, 0, KC)
            recursion(3)
            st_copy(1)
            v_mult(1, 0, KC)
            stores(1, STORE_GROUPS)

    nc.finalize()
    return nc


_CACHE = {}


def _get_program(betas):
    key = tuple(float(b) for b in betas)
    if key not in _CACHE:
        _CACHE[key] = build_program(key)
    return _CACHE[key]


def make_in_maps(x, W, b):
    """Shard x (fp16, transposed) across cores; replicate coefficients."""
    x = np.asarray(x, dtype=np.float32)
    W = np.asarray(W, dtype=np.float32)
    assert x.shape == (B, D) and W.shape == (L, D)

    x16 = x.astype(np.float16)
    # A = [ones, W0, W1, W2] as [P, KC*L]: a[p, c*L+i] = A[c*128+p, i]
    a_mat = np.concatenate([np.ones((D, 1), np.float32), W[: L - 1].T], axis=1)
    a_host = np.ascontiguousarray(
        a_mat.reshape(KC, P, L).transpose(1, 0, 2).reshape(P, KC * L)
    ).astype(np.float16)
    # W'' = [W; ones] as [L+1, D]
    wv_host = np.concatenate([W, np.ones((1, D), np.float32)], axis=0).astype(
        np.float16
    )
    return [
        {
            "xt": np.ascontiguousarray(x16[i * RB : (i + 1) * RB].T),
            "acoef": a_host,
            "wv": wv_host,
        }
        for i in range(N_CORES)
    ]


def kernel(**inputs) -> np.ndarray:
    x = np.asarray(inputs["x"], dtype=np.float32)
    W = np.asarray(inputs["W"], dtype=np.float32)
    b = np.asarray(inputs["b"], dtype=np.float32)

    betas = b.sum(axis=1, dtype=np.float64).astype(np.float32)
    nc = _get_program(betas)
    in_maps = make_in_maps(x, W, b)
    res = run_bass_kernel_spmd(nc, in_maps, list(range(N_CORES)))
    out = np.concatenate(
        [res.results[i]["out"].T for i in range(N_CORES)], axis=0
    ).astype(np.float32)

    bsum = b.sum(axis=0, dtype=np.float64).astype(np.float32)
    if np.any(bsum != 0.0):
        out = out + bsum[None, :]
    return out



# revision 10
# speedup vs baseline: 1.1084x; 1.0277x over previous
"""DCN CrossLayer kernel for Trainium2 (8 NeuronCores, batch-sharded).

Math: the reference loop
    cross = x
    for i in range(L):
        s_i   = sum(cross, axis=1)                  # (B, 1)
        cross = s_i * x * W[i] + b[i] + cross
collapses to
    out[b, k] = x[b, k] * (1 + sum_i s_i[b] * W[i, k]) + Bsum[k]
with
    u_i[b]  = sum_k x[b, k] * W[i, k]
    s_0[b]  = sum_k x[b, k]
    s_{i+1} = s_i * (1 + u_i) + beta_i,   beta_i = sum_k b[i, k]
    Bsum[k] = sum_i b[i, k]

Layout strategy (v2): the host uploads x PRE-TRANSPOSED and in fp16
(x^T: [D, rows], k on partitions).  This halves the input HBM bytes
(8 MiB -> 4 MiB per core) and removes all 128 big PE transposes the
natural layout needs: the k-contraction for [s_0, u_i] is a direct
PE matmul U = A^T @ x^T accumulated over 16 k-chunks, and the final
product is computed transposed, out^T = x^T * T^T with
T^T[k, b] = 1 + sum_i W[i, k] s'_i[b]  (one [5]-deep matmul per
[128 k x 512 b] chunk).  The host transposes the fp16 result back.

Per-core schedule: the 1024-row b-range splits into two 512-wide
halves.  Half 0's 16 k-chunk loads stream first, so its U/recursion/
T-multiply/store pipeline runs while half 1 loads; the final load
batches shrink to single chunks so U catches up during the stream.
The elementwise multiply (the only pass that must touch every output
element on a compute engine) is split DVE (reads T from PSUM
directly) / ACT-copy+GPSIMD / ACT-copy+DVE-fp16 so no single engine
paces the tail.  CoreSim DMA floor: 8 MiB @ 360 GB/s = 23.3 us.

Precision: fp16 x quantization ~2.4e-4, fp16 store ~2.4e-4, s'
chain ~3e-4 -> total rel err ~5e-4 (gate is 2e-2).
"""

import sys

sys.path.insert(0, "/opt/trn_rl_repo")

import numpy as np

import concourse.bacc as bacc
import concourse.tile as tile
from concourse import mybir
from concourse.bass_utils import run_bass_kernel_spmd
from concourse.masks import make_identity

N_CORES = 8
B, D, L = 8192, 2048, 4
RB = B // N_CORES            # 1024 batch rows per core
P = 128                      # partitions
KC = D // P                  # 16 k-chunks of 128
NQ = 4                       # b quarters per core
QW = RB // NQ                # 256 b columns per quarter
NSUB = QW // P               # 2 recursion subtiles per quarter

F32 = mybir.dt.float32
F16 = mybir.dt.float16
ADD = mybir.AluOpType.add
MULT = mybir.AluOpType.mult

NH = 2                       # b halves per core (multiply/store granularity)
HW = RB // NH                # 512 b columns per half

# Chunk ranges per load DMA within a quarter.
LOAD_BATCHES = ((0, 8), (8, 16))
# Chunk ranges per store DMA: 512-wide groups for half 0, 256-wide for q2/q3.
STORE_GROUPS_H = ((0, 4), (4, 8), (8, 12), (12, 16))
STORE_GROUPS_Q = ((0, 8), (8, 16))
# Multiply path per chunk: 'd' = DVE reads T from PSUM directly,
# 'g' = ACT copies T to fp16 SBUF + GPSIMD multiply.
PATHS = "dgdgdgdgdgdgdgdg"


def build_program(betas):
    """Build the per-core Bass program (same program on all 8 cores)."""
    nc = bacc.Bacc("TRN2", target_bir_lowering=False)

    xt_d = nc.dram_tensor("xt", [D, RB], F16, kind="ExternalInput")
    a_d = nc.dram_tensor("acoef", [P, KC * L], F16, kind="ExternalInput")
    wv_d = nc.dram_tensor("wv", [L + 1, D], F16, kind="ExternalInput")
    out_d = nc.dram_tensor("out", [D, RB], F16, kind="ExternalOutput")

    xt_t = xt_d.rearrange("(c p) b -> p c b", p=P)
    out_t = out_d.rearrange("(c p) b -> p c b", p=P)

    with tile.TileContext(nc) as tc:
        with (
            tc.tile_pool(name="consts", bufs=1) as consts,
            tc.tile_pool(name="xp", bufs=1) as xp,
            tc.tile_pool(name="op", bufs=1) as op,
            tc.tile_pool(name="smalls", bufs=1) as smalls,
            tc.tile_pool(name="tsbp", bufs=4) as tsbp,
            tc.tile_pool(name="u_ps", bufs=1, space="PSUM") as u_ps,
            tc.tile_pool(name="st_ps", bufs=1, space="PSUM") as st_ps,
            tc.tile_pool(name="t_ps", bufs=4, space="PSUM") as t_ps,
        ):
            # x loads lead on the SP ring; tiny consts ride SWDGE (no HWDGE
            # slot) so they only displace ~150ns of the x stream.
            xall = xp.tile([P, KC, RB], F16, tag="x")
            for q in range(NQ):
                qs = slice(q * QW, (q + 1) * QW)
                for lo, hi in LOAD_BATCHES:
                    nc.sync.dma_start(
                        out=xall[:, lo:hi, qs], in_=xt_t[:, lo:hi, qs]
                    )
                if q == 0:
                    a_sb = consts.tile([P, KC * L], F16)
                    nc.gpsimd.dma_start(out=a_sb, in_=a_d[:])
                    wv_sb = consts.tile([L + 1, D], F16)
                    nc.gpsimd.dma_start(out=wv_sb, in_=wv_d[:])
                    ident = consts.tile([P, P], F32)
                    make_identity(nc, ident)

            oall = op.tile([P, KC, RB], F16, tag="o")
            # U tiles: rows 0..3 hold the U accumulation; after the ACT
            # evacuation the same PSUM region is reused as the transpose
            # target for the natural-layout u (saves PSUM, and the WAR dep
            # is exactly the evacuation).
            u_tiles = [
                u_ps.tile([P, QW], F32, tag=f"u{q % 2}", name=f"u{q % 2}")
                for q in range(NQ)
            ]
            st_tiles = [None] * NQ

            def u_mms(q, lo, hi):
                """U^T accumulation matmuls for chunks [lo, hi) of quarter q."""
                qs = slice(q * QW, (q + 1) * QW)
                for c in range(lo, hi):
                    nc.tensor.matmul(
                        u_tiles[q][:L, :],
                        a_sb[:, c * L : (c + 1) * L],
                        xall[:, c, qs],
                        start=(c == 0),
                        stop=(c == KC - 1),
                    )

            # Per-half S' staging in SBUF: each quarter's recursion
            # transposes into its own small PSUM tile, then ACT evacuates
            # that [5, QW] slice into the half's shared [5, HW] fp16 tile.
            # Half-0 T matmuls read the full 512-wide tile; q2/q3 T matmuls
            # read their 256-wide slice (so q2 production doesn't wait on
            # q3's recursion).
            st_tiles = [
                smalls.tile([L + 1, HW], F16, tag=f"sth{h}", name=f"sth{h}")
                for h in range(NH)
            ]

            def recursion(q):
                """U -> S' for quarter q (into half q//2's stp tile).

                The [4, 256] U rows live on partitions 1..3, which compute
                engines cannot address individually (mod-32 base rule), so
                transpose to natural [128, sub, i] layout, run the chain on
                GPSIMD (keeps DVE free for multiplies), transpose back.
                """
                u_sb = smalls.tile([L, QW], F32, tag=f"usb{q}")
                nc.scalar.copy(u_sb, u_tiles[q][:L, :])
                un_ps = u_tiles[q]
                for s in range(NSUB):
                    nc.tensor.transpose(
                        un_ps[:, s * L : (s + 1) * L],
                        u_sb[:, s * P : (s + 1) * P],
                        ident[:L, :L],
                    )
                un_v = un_ps[:, : NSUB * L].rearrange("p (s l) -> p s l", s=NSUB)
                sn = smalls.tile([P, NSUB, L + 1], F32, tag=f"sn{q}")
                nc.gpsimd.memset(sn[:, :, L], 1.0)
                nc.scalar.copy(sn[:, :, 0], un_v[:, :, 0])
                if all(bt == 0.0 for bt in betas):
                    # ACT evacuates 1+u_i in one fused op; the chain is then
                    # three plain multiplies, which Pool supports (keeps DVE
                    # free; TensorScalarPtr is not a Pool instruction).
                    un1 = smalls.tile([P, NSUB, L - 1], F32, tag=f"un{q}")
                    nc.scalar.add(un1, un_v[:, :, 1:], 1.0)
                    for i in range(L - 1):
                        nc.gpsimd.tensor_mul(
                            sn[:, :, i + 1], sn[:, :, i], un1[:, :, i]
                        )
                else:
                    for i in range(L - 1):
                        nc.vector.scalar_tensor_tensor(
                            out=sn[:, :, i + 1],
                            in0=un_v[:, :, i + 1],
                            scalar=1.0,
                            in1=sn[:, :, i],
                            op0=ADD,
                            op1=MULT,
                        )
                        nc.vector.tensor_scalar_add(
                            sn[:, :, i + 1], sn[:, :, i + 1], float(betas[i])
                        )
                stp = st_ps.tile([L + 1, QW], F32, tag=f"stp{q % 2}", name="stp")
                for s in range(NSUB):
                    nc.tensor.transpose(
                        stp[:, s * P : (s + 1) * P], sn[:, s, :], ident
                    )
                off = (q % 2) * QW
                nc.scalar.copy(st_tiles[q // 2][:, off : off + QW], stp)

            def v_mult_h(h, lo, hi):
                """512-wide T^T matmul + multiply for chunks [lo, hi) of half h."""
                hs = slice(h * HW, (h + 1) * HW)
                for c in range(lo, hi):
                    tp = t_ps.tile([P, HW], F32, tag="t")
                    nc.tensor.matmul(
                        tp,
                        wv_sb[:, c * P : (c + 1) * P],
                        st_tiles[h],
                        start=True,
                        stop=True,
                    )
                    path = PATHS[c]
                    if path == "d":
                        nc.vector.tensor_mul(oall[:, c, hs], xall[:, c, hs], tp)
                    else:
                        tsb = tsbp.tile([P, HW], F16, tag="tsb")
                        nc.scalar.copy(tsb, tp)
                        nc.gpsimd.tensor_mul(oall[:, c, hs], xall[:, c, hs], tsb)

            def v_mult_q(q, lo, hi):
                """256-wide T^T matmul + multiply for chunks [lo, hi) of quarter q."""
                qs = slice(q * QW, (q + 1) * QW)
                off = (q % 2) * QW
                stq = st_tiles[q // 2][:, off : off + QW]
                for c in range(lo, hi):
                    tp = t_ps.tile([P, HW], F32, tag="t")
                    nc.tensor.matmul(
                        tp[:, :QW],
                        wv_sb[:, c * P : (c + 1) * P],
                        stq,
                        start=True,
                        stop=True,
                    )
                    path = PATHS[c]
                    if path == "d":
                        nc.vector.tensor_mul(
                            oall[:, c, qs], xall[:, c, qs], tp[:, :QW]
                        )
                    else:
                        tsb = tsbp.tile([P, HW], F16, tag="tsb")
                        nc.scalar.copy(tsb[:, :QW], tp[:, :QW])
                        nc.gpsimd.tensor_mul(
                            oall[:, c, qs], xall[:, c, qs], tsb[:, :QW]
                        )

            def stores(cols, width, groups):
                cs = slice(cols, cols + width)
                for lo, hi in groups:
                    nc.sync.dma_start(
                        out=out_t[:, lo:hi, cs],
                        in_=oall[:, lo:hi, cs],
                    )

            # Emission order sets scheduler PRIORITY and, effectively, each
            # engine's static program order — so instructions are emitted in
            # the order their inputs actually arrive at runtime.  Loads all
            # precede stores on the in-order SP queue, so the input streams
            # first and the store stream tails it gaplessly.  U/recursion
            # run per quarter (overlapping the load stream); half-0's
            # multiply pipeline is 512-wide (engine overheads amortize),
            # while q2/q3 run 256-wide so q2's production starts right
            # after its own recursion instead of waiting for q3's.
            u_mms(0, 0, 8)
            u_mms(0, 8, KC)
            recursion(0)
            u_mms(1, 0, 8)
            u_mms(1, 8, KC)
            recursion(1)
            v_mult_h(0, 0, 8)
            stores(0, HW, STORE_GROUPS_H[:2])
            u_mms(2, 0, 8)
            v_mult_h(0, 8, KC)
            stores(0, HW, STORE_GROUPS_H[2:])
            u_mms(2, 8, KC)
            recursion(2)
            u_mms(3, 0, 8)
            u_mms(3, 8, KC)
            recursion(3)
            v_mult_q(2, 0, KC)
            stores(2 * QW, QW, STORE_GROUPS_Q)
            v_mult_q(3, 0, KC)
            stores(3 * QW, QW, STORE_GROUPS_Q)

    nc.finalize()
    return nc


_CACHE = {}


def _get_program(betas):
    key = tuple(float(b) for b in betas)
    if key not in _CACHE:
        _CACHE[key] = build_program(key)
    return _CACHE[key]


def make_in_maps(x, W, b):
    """Shard x (fp16, transposed) across cores; replicate coefficients."""
    x = np.asarray(x, dtype=np.float32)
    W = np.asarray(W, dtype=np.float32)
    assert x.shape == (B, D) and W.shape == (L, D)

    x16 = x.astype(np.float16)
    # A = [ones, W0, W1, W2] as [P, KC*L]: a[p, c*L+i] = A[c*128+p, i]
    a_mat = np.concatenate([np.ones((D, 1), np.float32), W[: L - 1].T], axis=1)
    a_host = np.ascontiguousarray(
        a_mat.reshape(KC, P, L).transpose(1, 0, 2).reshape(P, KC * L)
    ).astype(np.float16)
    # W'' = [W; ones] as [L+1, D]
    wv_host = np.concatenate([W, np.ones((1, D), np.float32)], axis=0).astype(
        np.float16
    )
    return [
        {
            "xt": np.ascontiguousarray(x16[i * RB : (i + 1) * RB].T),
            "acoef": a_host,
            "wv": wv_host,
        }
        for i in range(N_CORES)
    ]


def kernel(**inputs) -> np.ndarray:
    x = np.asarray(inputs["x"], dtype=np.float32)
    W = np.asarray(inputs["W"], dtype=np.float32)
    b = np.asarray(inputs["b"], dtype=np.float32)

    betas = b.sum(axis=1, dtype=np.float64).astype(np.float32)
    nc = _get_program(betas)
    in_maps = make_in_maps(x, W, b)
    res = run_bass_kernel_spmd(nc, in_maps, list(range(N_CORES)))
    out = np.concatenate(
        [res.results[i]["out"].T for i in range(N_CORES)], axis=0
    ).astype(np.float32)

    bsum = b.sum(axis=0, dtype=np.float64).astype(np.float32)
    if np.any(bsum != 0.0):
        out = out + bsum[None, :]
    return out



# revision 11
# speedup vs baseline: 1.1135x; 1.0047x over previous
"""DCN CrossLayer kernel for Trainium2 (8 NeuronCores, batch-sharded).

Math: the reference loop
    cross = x
    for i in range(L):
        s_i   = sum(cross, axis=1)                  # (B, 1)
        cross = s_i * x * W[i] + b[i] + cross
collapses to
    out[b, k] = x[b, k] * (1 + sum_i s_i[b] * W[i, k]) + Bsum[k]
with
    u_i[b]  = sum_k x[b, k] * W[i, k]
    s_0[b]  = sum_k x[b, k]
    s_{i+1} = s_i * (1 + u_i) + beta_i,   beta_i = sum_k b[i, k]
    Bsum[k] = sum_i b[i, k]

Layout strategy (v2): the host uploads x PRE-TRANSPOSED and in fp16
(x^T: [D, rows], k on partitions).  This halves the input HBM bytes
(8 MiB -> 4 MiB per core) and removes all 128 big PE transposes the
natural layout needs: the k-contraction for [s_0, u_i] is a direct
PE matmul U = A^T @ x^T accumulated over 16 k-chunks, and the final
product is computed transposed, out^T = x^T * T^T with
T^T[k, b] = 1 + sum_i W[i, k] s'_i[b]  (one [5]-deep matmul per
[128 k x 512 b] chunk).  The host transposes the fp16 result back.

Per-core schedule: the 1024-row b-range splits into two 512-wide
halves.  Half 0's 16 k-chunk loads stream first, so its U/recursion/
T-multiply/store pipeline runs while half 1 loads; the final load
batches shrink to single chunks so U catches up during the stream.
The elementwise multiply (the only pass that must touch every output
element on a compute engine) is split DVE (reads T from PSUM
directly) / ACT-copy+GPSIMD / ACT-copy+DVE-fp16 so no single engine
paces the tail.  CoreSim DMA floor: 8 MiB @ 360 GB/s = 23.3 us.

Precision: fp16 x quantization ~2.4e-4, fp16 store ~2.4e-4, s'
chain ~3e-4 -> total rel err ~5e-4 (gate is 2e-2).
"""

import sys

sys.path.insert(0, "/opt/trn_rl_repo")

import numpy as np

import concourse.bacc as bacc
import concourse.tile as tile
from concourse import mybir
from concourse.bass_utils import run_bass_kernel_spmd
from concourse.masks import make_identity

N_CORES = 8
B, D, L = 8192, 2048, 4
RB = B // N_CORES            # 1024 batch rows per core
P = 128                      # partitions
KC = D // P                  # 16 k-chunks of 128
NQ = 4                       # b quarters per core
QW = RB // NQ                # 256 b columns per quarter
NSUB = QW // P               # 2 recursion subtiles per quarter

F32 = mybir.dt.float32
F16 = mybir.dt.float16
ADD = mybir.AluOpType.add
MULT = mybir.AluOpType.mult

NH = 2                       # b halves per core (multiply/store granularity)
HW = RB // NH                # 512 b columns per half

# Chunk ranges per load DMA within a quarter.
LOAD_BATCHES = ((0, 8), (8, 16))
# Chunk ranges per store DMA: 512-wide groups for half 0, 256-wide for q2/q3.
STORE_GROUPS_H = ((0, 4), (4, 8), (8, 12), (12, 16))
STORE_GROUPS_Q = ((0, 8), (8, 16))
# Multiply path per chunk: 'd' = DVE reads T from PSUM directly,
# 'g' = ACT copies T to fp16 SBUF + GPSIMD multiply.
PATHS = "dgdgdgdgdgdgdgdg"


def build_program(betas):
    """Build the per-core Bass program (same program on all 8 cores)."""
    nc = bacc.Bacc("TRN2", target_bir_lowering=False)

    xt_d = nc.dram_tensor("xt", [D, RB], F16, kind="ExternalInput")
    a_d = nc.dram_tensor("acoef", [P, KC * L], F16, kind="ExternalInput")
    wv_d = nc.dram_tensor("wv", [L + 1, D], F16, kind="ExternalInput")
    out_d = nc.dram_tensor("out", [D, RB], F16, kind="ExternalOutput")

    xt_t = xt_d.rearrange("(c p) b -> p c b", p=P)
    out_t = out_d.rearrange("(c p) b -> p c b", p=P)

    with tile.TileContext(nc) as tc:
        with (
            tc.tile_pool(name="consts", bufs=1) as consts,
            tc.tile_pool(name="xp", bufs=1) as xp,
            tc.tile_pool(name="op", bufs=1) as op,
            tc.tile_pool(name="smalls", bufs=1) as smalls,
            tc.tile_pool(name="tsbp", bufs=4) as tsbp,
            tc.tile_pool(name="u_ps", bufs=1, space="PSUM") as u_ps,
            tc.tile_pool(name="st_ps", bufs=1, space="PSUM") as st_ps,
            tc.tile_pool(name="t_ps", bufs=4, space="PSUM") as t_ps,
        ):
            # x loads lead on the SP ring; tiny consts ride SWDGE (no HWDGE
            # slot) so they only displace ~150ns of the x stream.
            xall = xp.tile([P, KC, RB], F16, tag="x")
            for q in range(NQ):
                qs = slice(q * QW, (q + 1) * QW)
                for lo, hi in LOAD_BATCHES:
                    nc.sync.dma_start(
                        out=xall[:, lo:hi, qs], in_=xt_t[:, lo:hi, qs]
                    )
                if q == 0:
                    a_sb = consts.tile([P, KC * L], F16)
                    nc.gpsimd.dma_start(out=a_sb, in_=a_d[:])
                    wv_sb = consts.tile([L + 1, D], F16)
                    nc.gpsimd.dma_start(out=wv_sb, in_=wv_d[:])
                    ident = consts.tile([P, P], F32)
                    make_identity(nc, ident)

            oall = op.tile([P, KC, RB], F16, tag="o")
            # U tiles: rows 0..3 hold the U accumulation; after the ACT
            # evacuation the same PSUM region is reused as the transpose
            # target for the natural-layout u (saves PSUM, and the WAR dep
            # is exactly the evacuation).
            u_tiles = [
                u_ps.tile([P, QW], F32, tag=f"u{q % 2}", name=f"u{q % 2}")
                for q in range(NQ)
            ]
            st_tiles = [None] * NQ

            def u_mms(q, lo, hi):
                """U^T accumulation matmuls for chunks [lo, hi) of quarter q."""
                qs = slice(q * QW, (q + 1) * QW)
                for c in range(lo, hi):
                    nc.tensor.matmul(
                        u_tiles[q][:L, :],
                        a_sb[:, c * L : (c + 1) * L],
                        xall[:, c, qs],
                        start=(c == 0),
                        stop=(c == KC - 1),
                    )

            # Per-half S' staging in SBUF: each quarter's recursion
            # transposes into its own small PSUM tile, then ACT evacuates
            # that [5, QW] slice into the half's shared [5, HW] fp16 tile.
            # Half-0 T matmuls read the full 512-wide tile; q2/q3 T matmuls
            # read their 256-wide slice (so q2 production doesn't wait on
            # q3's recursion).
            st_tiles = [
                smalls.tile([L + 1, HW], F16, tag=f"sth{h}", name=f"sth{h}")
                for h in range(NH)
            ]

            def recursion(q):
                """U -> S' for quarter q (into half q//2's stp tile).

                The [4, 256] U rows live on partitions 1..3, which compute
                engines cannot address individually (mod-32 base rule), so
                transpose to natural [128, sub, i] layout, run the chain on
                GPSIMD (keeps DVE free for multiplies), transpose back.
                """
                u_sb = smalls.tile([L, QW], F32, tag=f"usb{q}")
                nc.scalar.copy(u_sb, u_tiles[q][:L, :])
                un_ps = u_tiles[q]
                for s in range(NSUB):
                    nc.tensor.transpose(
                        un_ps[:, s * L : (s + 1) * L],
                        u_sb[:, s * P : (s + 1) * P],
                        ident[:L, :L],
                    )
                un_v = un_ps[:, : NSUB * L].rearrange("p (s l) -> p s l", s=NSUB)
                sn = smalls.tile([P, NSUB, L + 1], F32, tag=f"sn{q}")
                nc.gpsimd.memset(sn[:, :, L], 1.0)
                nc.scalar.copy(sn[:, :, 0], un_v[:, :, 0])
                if all(bt == 0.0 for bt in betas):
                    # ACT evacuates 1+u_i in one fused op; the chain is then
                    # three plain multiplies, which Pool supports (keeps DVE
                    # free; TensorScalarPtr is not a Pool instruction).
                    un1 = smalls.tile([P, NSUB, L - 1], F32, tag=f"un{q}")
                    nc.scalar.add(un1, un_v[:, :, 1:], 1.0)
                    for i in range(L - 1):
                        nc.gpsimd.tensor_mul(
                            sn[:, :, i + 1], sn[:, :, i], un1[:, :, i]
                        )
                else:
                    for i in range(L - 1):
                        nc.vector.scalar_tensor_tensor(
                            out=sn[:, :, i + 1],
                            in0=un_v[:, :, i + 1],
                            scalar=1.0,
                            in1=sn[:, :, i],
                            op0=ADD,
                            op1=MULT,
                        )
                        nc.vector.tensor_scalar_add(
                            sn[:, :, i + 1], sn[:, :, i + 1], float(betas[i])
                        )
                stp = st_ps.tile([L + 1, QW], F32, tag=f"stp{q % 2}", name="stp")
                for s in range(NSUB):
                    nc.tensor.transpose(
                        stp[:, s * P : (s + 1) * P], sn[:, s, :], ident
                    )
                off = (q % 2) * QW
                nc.scalar.copy(st_tiles[q // 2][:, off : off + QW], stp)

            def v_mult_h(h, lo, hi):
                """512-wide T^T matmul + multiply for chunks [lo, hi) of half h."""
                hs = slice(h * HW, (h + 1) * HW)
                for c in range(lo, hi):
                    tp = t_ps.tile([P, HW], F32, tag="t")
                    nc.tensor.matmul(
                        tp,
                        wv_sb[:, c * P : (c + 1) * P],
                        st_tiles[h],
                        start=True,
                        stop=True,
                    )
                    path = PATHS[c]
                    if path == "d":
                        nc.vector.tensor_mul(oall[:, c, hs], xall[:, c, hs], tp)
                    else:
                        tsb = tsbp.tile([P, HW], F16, tag="tsb")
                        nc.scalar.copy(tsb, tp)
                        nc.gpsimd.tensor_mul(oall[:, c, hs], xall[:, c, hs], tsb)

            def v_mult_q(q, lo, hi):
                """256-wide T^T matmul + multiply for chunks [lo, hi) of quarter q."""
                qs = slice(q * QW, (q + 1) * QW)
                off = (q % 2) * QW
                stq = st_tiles[q // 2][:, off : off + QW]
                for c in range(lo, hi):
                    tp = t_ps.tile([P, HW], F32, tag="t")
                    nc.tensor.matmul(
                        tp[:, :QW],
                        wv_sb[:, c * P : (c + 1) * P],
                        stq,
                        start=True,
                        stop=True,
                    )
                    path = PATHS[c]
                    if path == "d":
                        nc.vector.tensor_mul(
                            oall[:, c, qs], xall[:, c, qs], tp[:, :QW]
                        )
                    else:
                        tsb = tsbp.tile([P, HW], F16, tag="tsb")
                        nc.scalar.copy(tsb[:, :QW], tp[:, :QW])
                        nc.gpsimd.tensor_mul(
                            oall[:, c, qs], xall[:, c, qs], tsb[:, :QW]
                        )

            def stores(cols, width, groups):
                cs = slice(cols, cols + width)
                for lo, hi in groups:
                    nc.sync.dma_start(
                        out=out_t[:, lo:hi, cs],
                        in_=oall[:, lo:hi, cs],
                    )

            # Emission order sets scheduler PRIORITY and, effectively, each
            # engine's static program order — so instructions are emitted in
            # the order their inputs actually arrive at runtime.  Loads all
            # precede stores on the in-order SP queue, so the input streams
            # first and the store stream tails it gaplessly.  U/recursion
            # run per quarter (overlapping the load stream); half-0's
            # multiply pipeline is 512-wide (engine overheads amortize),
            # while q2/q3 run 256-wide so q2's production starts right
            # after its own recursion instead of waiting for q3's.
            u_mms(0, 0, 8)
            u_mms(0, 8, KC)
            recursion(0)
            u_mms(1, 0, 8)
            u_mms(1, 8, KC)
            recursion(1)
            v_mult_h(0, 0, 8)
            stores(0, HW, STORE_GROUPS_H[:2])
            u_mms(2, 0, 8)
            u_mms(2, 8, KC)
            recursion(2)
            u_mms(3, 0, 8)
            v_mult_h(0, 8, 12)
            stores(0, HW, STORE_GROUPS_H[2:3])
            u_mms(3, 8, KC)
            v_mult_h(0, 12, KC)
            stores(0, HW, STORE_GROUPS_H[3:])
            recursion(3)
            v_mult_q(2, 0, KC)
            stores(2 * QW, QW, STORE_GROUPS_Q)
            v_mult_q(3, 0, KC)
            stores(3 * QW, QW, ((0, 4), (4, 8), (8, 12), (12, 16)))

    nc.finalize()
    return nc


_CACHE = {}


def _get_program(betas):
    key = tuple(float(b) for b in betas)
    if key not in _CACHE:
        _CACHE[key] = build_program(key)
    return _CACHE[key]


def make_in_maps(x, W, b):
    """Shard x (fp16, transposed) across cores; replicate coefficients."""
    x = np.asarray(x, dtype=np.float32)
    W = np.asarray(W, dtype=np.float32)
    assert x.shape == (B, D) and W.shape == (L, D)

    x16 = x.astype(np.float16)
    # A = [ones, W0, W1, W2] as [P, KC*L]: a[p, c*L+i] = A[c*128+p, i]
    a_mat = np.concatenate([np.ones((D, 1), np.float32), W[: L - 1].T], axis=1)
    a_host = np.ascontiguousarray(
        a_mat.reshape(KC, P, L).transpose(1, 0, 2).reshape(P, KC * L)
    ).astype(np.float16)
    # W'' = [W; ones] as [L+1, D]
    wv_host = np.concatenate([W, np.ones((1, D), np.float32)], axis=0).astype(
        np.float16
    )
    return [
        {
            "xt": np.ascontiguousarray(x16[i * RB : (i + 1) * RB].T),
            "acoef": a_host,
            "wv": wv_host,
        }
        for i in range(N_CORES)
    ]


def kernel(**inputs) -> np.ndarray:
    x = np.asarray(inputs["x"], dtype=np.float32)
    W = np.asarray(inputs["W"], dtype=np.float32)
    b = np.asarray(inputs["b"], dtype=np.float32)

    betas = b.sum(axis=1, dtype=np.float64).astype(np.float32)
    nc = _get_program(betas)
    in_maps = make_in_maps(x, W, b)
    res = run_bass_kernel_spmd(nc, in_maps, list(range(N_CORES)))
    out = np.concatenate(
        [res.results[i]["out"].T for i in range(N_CORES)], axis=0
    ).astype(np.float32)

    bsum = b.sum(axis=0, dtype=np.float64).astype(np.float32)
    if np.any(bsum != 0.0):
        out = out + bsum[None, :]
    return out

